# revision 18
# baseline (speedup 1.0000x reference)
"""Trainium2 Bass kernel for nn_LowPassMAELoss.

reference:  mean(|lfilter(output) - lfilter(target)|)  with a 6th-order
Butterworth low-pass IIR (direct-form II transposed, f32) along T.

Approach:
  * lfilter is linear  =>  lfilter(o) - lfilter(t) = lfilter(o - t).
  * The IIR is stable (max pole radius 0.878); its impulse response decays
    below 1e-14 (summed) past 256 taps, so a 256-tap FIR is exact at f32
    precision.
  * The FIR becomes a block-Toeplitz matmul on the tensor engine:
    with 128-sample blocks x_i,  y_i = H0 @ x_i + H1 @ x_{i-1}
    where Hs[r, c] = h[128 s + r - c]  (h = impulse response).
  * Sharding: 96 waveforms -> 12 per core across 8 cores (data parallel).
    Each core returns per-(partition, group) partial sums of |y|; the host
    reduces and divides.

Dataflow per core (12 waveforms x 131072 samples):
  DMA [128, 8, 128] staging (partition p holds blocks 128u + p)
  -> DVE subtract -> PE transpose (128x128 tiles) -> ACT copy PSUM->SBUF
  -> PE conv matmuls (2 accumulating matmuls per 512-block group, fp32r)
  -> DVE abs-sum reduce from PSUM -> acc[128, 24] -> DMA out.
"""

import sys

if "/opt/trn_rl_repo" not in sys.path:
    sys.path.insert(0, "/opt/trn_rl_repo")

from contextlib import ExitStack

import numpy as np

import concourse.bacc as bacc
import concourse.tile as tile
from concourse import masks, mybir
from concourse import bass_utils

F32 = mybir.dt.float32
F32R = mybir.dt.float32r

B, T = 96, 131072
NCORES = 8
WPC = B // NCORES          # waveforms per core
P = 128                    # partitions / block size
U = T // (P * P)           # 8 sub-tiles per waveform
NBLK = T // P              # 1024 blocks per waveform
GRP = 512                  # output blocks per conv matmul group
NGRP = NBLK // GRP         # 2 groups per waveform
KT = 256                   # FIR taps

CUTOFF_FREQ = 4000.0
SAMPLE_RATE = 48000.0
FILTER_ORDER = 6

# Which dtype the conv matmuls run in: "f32r" (full speed) or "f32" (4x slower,
# bit-exact fp32).
CONV_DTYPE = "f32r"


def _butter_lowpass(order, wn):
    fs = 2.0
    warped = 2.0 * fs * np.tan(np.pi * wn / fs)
    m = np.arange(-order + 1, order, 2)
    p = -np.exp(1j * np.pi * m / (2 * order))
    p = warped * p
    k = warped ** order
    fs2 = 2.0 * fs
    pz = (fs2 + p) / (fs2 - p)
    kz = k * np.real(1.0 / np.prod(fs2 - p))
    b = (kz * np.poly(-np.ones(order))).real
    a = np.poly(pz).real
    return b.astype(np.float32), a.astype(np.float32)


def _impulse_response(n):
    """First n taps of the reference filter's impulse response, in float64
    (coefficients quantized to f32 exactly as the reference does)."""
    b32, a32 = _butter_lowpass(FILTER_ORDER, CUTOFF_FREQ / (0.5 * SAMPLE_RATE))
    b = b32.astype(np.float64)
    a = a32.astype(np.float64)
    order = len(b) - 1
    z = np.zeros(order)
    h = np.zeros(n)
    x = np.zeros(n)
    x[0] = 1.0
    b0, bi, ai = b[0], b[1:], a[1:]
    for t in range(n):
        y = b0 * x[t] + z[0]
        z = np.concatenate([z[1:], [0.0]]) + bi * x[t] - ai * y
        h[t] = y
    return h


def _h_matrices():
    """hm[c, s, r] = h[128 s + r - c] — the pre-transposed stationary operands
    (lhsT) for the two conv matmuls."""
    h = _impulse_response(KT)
    r = np.arange(P)[None, :]
    c = np.arange(P)[:, None]
    hm = np.zeros((P, 2, P), np.float32)
    for s in range(2):
        j = 128 * s + r - c
        valid = (j >= 0) & (j < KT)
        hm[:, s, :] = np.where(valid, h[np.clip(j, 0, KT - 1)], 0.0).astype(
            np.float32
        )
    return hm


def _build_program():
    conv_dt = F32R if CONV_DTYPE == "f32r" else F32
    nc = bacc.Bacc("TRN2", target_bir_lowering=False, debug=False)

    # output and target packed into one tensor so each waveform is ONE 1MB
    # DMA -> the subtract has a single DMA-semaphore wait (the TensorTensor
    # ISA struct only has room for one sync wait).
    x_both = nc.dram_tensor(
        "x_both", [WPC, 2, T], F32, kind="ExternalInput"
    ).ap()
    hmat = nc.dram_tensor("hmat", [P, 2, P], conv_dt, kind="ExternalInput").ap()
    zpad = nc.dram_tensor("zpad", [P, 1], conv_dt, kind="ExternalInput").ap()
    acc = nc.dram_tensor(
        "acc", [P, WPC * NGRP], F32, kind="ExternalOutput"
    ).ap()

    # [w, p, k, u, c]: element = x[w, k, u*16384 + p*128 + c] (block 128u + p)
    xb_v = x_both.rearrange("w k (u p c) -> w p k u c", u=U, p=P, c=P)

    with ExitStack() as ctx:
        tc = ctx.enter_context(tile.TileContext(nc))
        singles = ctx.enter_context(tc.tile_pool(name="singles", bufs=1))
        stage = ctx.enter_context(tc.tile_pool(name="stage", bufs=2))
        dpool = ctx.enter_context(tc.tile_pool(name="dpool", bufs=2))
        xtp = ctx.enter_context(tc.tile_pool(name="xtp", bufs=2))
        tpsum = ctx.enter_context(
            tc.tile_pool(name="tpsum", bufs=4, space="PSUM")
        )
        ypsum = ctx.enter_context(
            tc.tile_pool(name="ypsum", bufs=2, space="PSUM")
        )

        # Constants are "pre-touched" by DVE copies so that consumers (PE)
        # only ever need the DVE semaphore they already wait on for data —
        # several ISA structs (TT, LW) have a single sync-wait slot.
        hmat_sb = singles.tile([P, 2, P], conv_dt)
        nc.sync.dma_start(hmat_sb, hmat)
        hmat_c = singles.tile([P, 2, P], conv_dt)
        nc.vector.tensor_copy(hmat_c, hmat_sb)
        ident_g = singles.tile([P, P], F32)
        masks.make_identity(nc, ident_g)
        ident = singles.tile([P, P], F32)
        nc.vector.tensor_copy(ident, ident_g)
        acc_sb = singles.tile([P, WPC * NGRP], F32)

        h0 = hmat_c[:, 0, :]
        h1 = hmat_c[:, 1, :]

        for w in range(WPC):
            big = stage.tile([P, 2, U, P], F32)
            nc.sync.dma_start(big, xb_v[w])

            d_t = dpool.tile([P, U, P], F32)
            nc.vector.tensor_sub(d_t, big[:, 0], big[:, 1])

            # xt[:, 0] = zero pad; xt[:, 1 + i] = transposed block i
            xt = xtp.tile([P, 1 + NBLK], conv_dt)
            nc.sync.dma_start(xt[:, 0:1], zpad)
            for u in range(U):
                tp = tpsum.tile([P, P], F32)
                nc.tensor.transpose(tp, d_t[:, u, :], ident)
                nc.scalar.copy(xt[:, 1 + P * u : 1 + P * (u + 1)], tp)

            for g in range(NGRP):
                yp = ypsum.tile([P, GRP], F32)
                rhs0 = xt[:, 1 + GRP * g : 1 + GRP * (g + 1)]
                rhs1 = xt[:, GRP * g : GRP * g + GRP]
                nc.tensor.matmul(yp, h0, rhs0, start=True, stop=False)
                nc.tensor.matmul(yp, h1, rhs1, start=False, stop=True)
                nc.vector.tensor_reduce(
                    acc_sb[:, w * NGRP + g : w * NGRP + g + 1],
                    yp,
                    axis=mybir.AxisListType.X,
                    op=mybir.AluOpType.add,
                    apply_absolute_value=True,
                )

        nc.sync.dma_start(acc, acc_sb)

    # Splits multi-sem waits into event semaphores (TRN2 allows only one
    # embedded wait per instruction), among other legalization passes.
    nc.compile()
    return nc


_CACHE = {}


def _get_program():
    if "nc" not in _CACHE:
        _CACHE["nc"] = _build_program()
        _CACHE["hmat"] = _h_matrices()
    return _CACHE["nc"], _CACHE["hmat"]


def _run(output, target, trace=False):
    """Run on the 8 NeuronCores; returns (result_scalar, BassKernelResults)."""
    nc, hm = _get_program()
    output = np.asarray(output, dtype=np.float32)
    target = np.asarray(target, dtype=np.float32)
    both = np.stack([output, target], axis=1)  # [B, 2, T] contiguous
    in_maps = []
    for i in range(NCORES):
        sl = slice(i * WPC, (i + 1) * WPC)
        in_maps.append(
            {
                "x_both": np.ascontiguousarray(both[sl]),
                "hmat": hm,
                "zpad": np.zeros((P, 1), np.float32),
            }
        )
    res = bass_utils.run_bass_kernel_spmd(
        nc, in_maps, core_ids=list(range(NCORES)), trace=trace
    )
    total = 0.0
    for r in res.results:
        total += r["acc"].astype(np.float64).sum()
    value = np.float32(total / (B * T))
    return value, res


def kernel(output, target):
    value, _ = _run(output, target)
    return np.asarray(value, dtype=np.float32)


# revision 29
# speedup vs baseline: 1.3088x; 1.3088x over previous
"""Trainium2 Bass kernel for nn_LowPassMAELoss.

reference:  mean(|lfilter(output) - lfilter(target)|)  with a 6th-order
Butterworth low-pass IIR (direct-form II transposed, f32) along T.

Approach:
  * lfilter is linear  =>  lfilter(o) - lfilter(t) = lfilter(o - t).
  * The IIR is stable (max pole radius 0.878); its impulse response decays
    below 1e-14 (summed) past 256 taps, so a 256-tap FIR is exact at f32
    precision.
  * The FIR becomes a block-Toeplitz matmul on the tensor engine:
    with 128-sample blocks x_i,  y_i = H0 @ x_i + H1 @ x_{i-1}
    where Hs[r, c] = h[128 s + r - c]  (h = impulse response).
  * Sharding: 96 waveforms -> 12 per core across 8 cores (data parallel).
    Each core returns per-(partition, group) partial sums of |y|; the host
    reduces and divides.

Dataflow per core (12 waveforms x 131072 samples):
  DMA [128, 8, 128] staging (partition p holds blocks 128u + p)
  -> DVE subtract -> PE transpose (128x128 tiles) -> ACT copy PSUM->SBUF
  -> PE conv matmuls (2 accumulating matmuls per 512-block group, fp32r)
  -> DVE abs-sum reduce from PSUM -> acc[128, 24] -> DMA out.
"""

import sys

if "/opt/trn_rl_repo" not in sys.path:
    sys.path.insert(0, "/opt/trn_rl_repo")

from contextlib import ExitStack

import numpy as np

import concourse.bacc as bacc
import concourse.tile as tile
from concourse import masks, mybir
from concourse import bass_utils

F32 = mybir.dt.float32
F32R = mybir.dt.float32r

B, T = 96, 131072
NCORES = 8
WPC = B // NCORES          # waveforms per core
P = 128                    # partitions / block size
U = T // (P * P)           # 8 sub-tiles per waveform
NBLK = T // P              # 1024 blocks per waveform
GRP = 512                  # output blocks per conv matmul group
NGRP = NBLK // GRP         # 2 groups per waveform
KT = 256                   # FIR taps

CUTOFF_FREQ = 4000.0
SAMPLE_RATE = 48000.0
FILTER_ORDER = 6

# Which dtype the conv matmuls run in: "f32r" (full speed) or "f32" (4x slower,
# bit-exact fp32).
CONV_DTYPE = "f32r"


def _butter_lowpass(order, wn):
    fs = 2.0
    warped = 2.0 * fs * np.tan(np.pi * wn / fs)
    m = np.arange(-order + 1, order, 2)
    p = -np.exp(1j * np.pi * m / (2 * order))
    p = warped * p
    k = warped ** order
    fs2 = 2.0 * fs
    pz = (fs2 + p) / (fs2 - p)
    kz = k * np.real(1.0 / np.prod(fs2 - p))
    b = (kz * np.poly(-np.ones(order))).real
    a = np.poly(pz).real
    return b.astype(np.float32), a.astype(np.float32)


def _impulse_response(n):
    """First n taps of the reference filter's impulse response, in float64
    (coefficients quantized to f32 exactly as the reference does)."""
    b32, a32 = _butter_lowpass(FILTER_ORDER, CUTOFF_FREQ / (0.5 * SAMPLE_RATE))
    b = b32.astype(np.float64)
    a = a32.astype(np.float64)
    order = len(b) - 1
    z = np.zeros(order)
    h = np.zeros(n)
    x = np.zeros(n)
    x[0] = 1.0
    b0, bi, ai = b[0], b[1:], a[1:]
    for t in range(n):
        y = b0 * x[t] + z[0]
        z = np.concatenate([z[1:], [0.0]]) + bi * x[t] - ai * y
        h[t] = y
    return h


def _h_matrices():
    """hm[c, s, r] = h[128 s + r - c] — the pre-transposed stationary operands
    (lhsT) for the two conv matmuls."""
    h = _impulse_response(KT)
    r = np.arange(P)[None, :]
    c = np.arange(P)[:, None]
    hm = np.zeros((P, 2, P), np.float32)
    for s in range(2):
        j = 128 * s + r - c
        valid = (j >= 0) & (j < KT)
        hm[:, s, :] = np.where(valid, h[np.clip(j, 0, KT - 1)], 0.0).astype(
            np.float32
        )
    return hm


def _build_program(unroll=1, stages="full"):
    """unroll > 1 replicates the whole body (same data) inside one NEFF —
    used only for steady-state throughput measurement in bench.py.
    stages: "full" | "dma" (DMA+subtract only) | "noconv" (through copies)."""
    conv_dt = F32R if CONV_DTYPE == "f32r" else F32
    nc = bacc.Bacc("TRN2", target_bir_lowering=False, debug=False)

    # output and target packed into one tensor so each waveform is ONE 1MB
    # DMA -> the subtract has a single DMA-semaphore wait (the TensorTensor
    # ISA struct only has room for one sync wait).
    x_both = nc.dram_tensor(
        "x_both", [WPC, 2, T], F32, kind="ExternalInput"
    ).ap()
    hmat = nc.dram_tensor("hmat", [P, 2, P], conv_dt, kind="ExternalInput").ap()
    zpad = nc.dram_tensor("zpad", [P, 1], conv_dt, kind="ExternalInput").ap()
    acc = nc.dram_tensor(
        "acc", [P, WPC * NGRP], F32, kind="ExternalOutput"
    ).ap()

    # Strip layout: partition p holds a contiguous 4KB run x[w, k, 1024p+f]
    # (one large DMA descriptor per partition per tensor). Block index of
    # 128-sample block u within partition p is B = 8p + u; the strided ACT
    # copy after the transpose restores natural block order in xt.
    FPW = T // P  # 1024 samples per partition per waveform
    xb_v = x_both.rearrange("w k (p f) -> w p k f", p=P, f=FPW)

    with ExitStack() as ctx:
        tc = ctx.enter_context(tile.TileContext(nc))
        singles = ctx.enter_context(tc.tile_pool(name="singles", bufs=1))
        stage = ctx.enter_context(tc.tile_pool(name="stage", bufs=2))
        dpool = ctx.enter_context(tc.tile_pool(name="dpool", bufs=2))
        xtp = ctx.enter_context(tc.tile_pool(name="xtp", bufs=2))
        tpsum = ctx.enter_context(
            tc.tile_pool(name="tpsum", bufs=4, space="PSUM")
        )
        ypsum = ctx.enter_context(
            tc.tile_pool(name="ypsum", bufs=2, space="PSUM")
        )

        # Constants are "pre-touched" by DVE copies so that consumers (PE)
        # only ever need the DVE semaphore they already wait on for data —
        # several ISA structs (TT, LW) have a single sync-wait slot.
        hmat_sb = singles.tile([P, 2, P], conv_dt)
        nc.sync.dma_start(hmat_sb, hmat)
        hmat_c = singles.tile([P, 2, P], conv_dt)
        nc.vector.tensor_copy(hmat_c, hmat_sb)
        ident_g = singles.tile([P, P], F32)
        masks.make_identity(nc, ident_g)
        ident = singles.tile([P, P], conv_dt)
        nc.vector.tensor_copy(ident, ident_g)
        acc_sb = singles.tile([P, WPC * NGRP], F32)

        h0 = hmat_c[:, 0, :]
        h1 = hmat_c[:, 1, :]

        for w0 in range(WPC * unroll):
            w = w0 % WPC
            big = stage.tile([P, 2, FPW], F32)
            nc.sync.dma_start(big, xb_v[w])

            d_t = dpool.tile([P, FPW], conv_dt)
            nc.vector.tensor_sub(d_t, big[:, 0], big[:, 1])

            if stages == "dma":
                for g in range(NGRP):
                    nc.vector.tensor_reduce(
                        acc_sb[:, w * NGRP + g : w * NGRP + g + 1],
                        d_t[:, GRP * g : GRP * (g + 1)],
                        axis=mybir.AxisListType.X,
                        op=mybir.AluOpType.add,
                        apply_absolute_value=True,
                    )
                continue

            # xt[:, 0] = zero pad; xt[:, 1 + B] = transposed block B
            xt = xtp.tile([P, 1 + NBLK], conv_dt)
            nc.sync.dma_start(xt[:, 0:1], zpad)
            # view of xt[:, 1:] with free dims [u (stride 1), p (stride 8)]
            xt_blk = xt[:, 1 : 1 + NBLK].rearrange(
                "c (p uu) -> c uu p", uu=U, p=P
            )
            for h in range(2):
                tp = tpsum.tile([P, 4, P], conv_dt)
                for j in range(4):
                    u = 4 * h + j
                    nc.tensor.transpose(
                        tp[:, j, :], d_t[:, P * u : P * (u + 1)], ident
                    )
                # psum [c, j, p] -> xt columns 1 + 8p + 4h + j
                nc.scalar.copy(xt_blk[:, 4 * h : 4 * h + 4, :], tp)

            if stages == "noconv":
                for g in range(NGRP):
                    nc.vector.tensor_reduce(
                        acc_sb[:, w * NGRP + g : w * NGRP + g + 1],
                        xt[:, 1 + GRP * g : 1 + GRP * (g + 1)],
                        axis=mybir.AxisListType.X,
                        op=mybir.AluOpType.add,
                        apply_absolute_value=True,
                    )
                continue

            for g in range(NGRP):
                yp = ypsum.tile([P, GRP], F32)
                rhs0 = xt[:, 1 + GRP * g : 1 + GRP * (g + 1)]
                rhs1 = xt[:, GRP * g : GRP * g + GRP]
                nc.tensor.matmul(yp, h0, rhs0, start=True, stop=False)
                nc.tensor.matmul(yp, h1, rhs1, start=False, stop=True)
                nc.vector.tensor_reduce(
                    acc_sb[:, w * NGRP + g : w * NGRP + g + 1],
                    yp,
                    axis=mybir.AxisListType.X,
                    op=mybir.AluOpType.add,
                    apply_absolute_value=True,
                )

        nc.sync.dma_start(acc, acc_sb)

    # Splits multi-sem waits into event semaphores (TRN2 allows only one
    # embedded wait per instruction), among other legalization passes.
    nc.compile()
    return nc


_CACHE = {}


def _get_program(unroll=1, stages="full"):
    key = ("nc", unroll, stages)
    if key not in _CACHE:
        _CACHE[key] = _build_program(unroll, stages)
    if "hmat" not in _CACHE:
        _CACHE["hmat"] = _h_matrices()
    return _CACHE[key], _CACHE["hmat"]


def _run(output, target, trace=False):
    """Run on the 8 NeuronCores; returns (result_scalar, BassKernelResults)."""
    nc, hm = _get_program()
    output = np.asarray(output, dtype=np.float32)
    target = np.asarray(target, dtype=np.float32)
    both = np.stack([output, target], axis=1)  # [B, 2, T] contiguous
    in_maps = []
    for i in range(NCORES):
        sl = slice(i * WPC, (i + 1) * WPC)
        in_maps.append(
            {
                "x_both": np.ascontiguousarray(both[sl]),
                "hmat": hm,
                "zpad": np.zeros((P, 1), np.float32),
            }
        )
    res = bass_utils.run_bass_kernel_spmd(
        nc, in_maps, core_ids=list(range(NCORES)), trace=trace
    )
    total = 0.0
    for r in res.results:
        total += r["acc"].astype(np.float64).sum()
    value = np.float32(total / (B * T))
    return value, res


def kernel(output, target):
    value, _ = _run(output, target)
    return np.asarray(value, dtype=np.float32)


# revision 30
# speedup vs baseline: 27086.5546x; 20695.8510x over previous
"""Trainium2 Bass kernel for nn_LowPassMAELoss.

reference:  mean(|lfilter(output) - lfilter(target)|)  with a 6th-order
Butterworth low-pass IIR (direct-form II transposed, f32) along T.

Approach:
  * lfilter is linear  =>  lfilter(o) - lfilter(t) = lfilter(o - t).
  * The IIR is stable (max pole radius 0.878); its impulse response decays
    below 1e-14 (summed) past 256 taps, so a 256-tap FIR is exact at f32
    precision.
  * The FIR becomes a block-Toeplitz matmul on the tensor engine:
    with 128-sample blocks x_i,  y_i = H0 @ x_i + H1 @ x_{i-1}
    where Hs[r, c] = h[128 s + r - c]  (h = impulse response).
  * Sharding: 96 waveforms -> 12 per core across 8 cores (data parallel).
    Each core returns per-(partition, group) partial sums of |y|; the host
    reduces and divides.

Dataflow per core (12 waveforms x 131072 samples):
  one 1MB DMA per waveform, strip layout (partition p = contiguous 4KB run,
  so DMA descriptors are large) -> DVE subtract -> PE transpose (fp32r,
  128x128 tiles, 4 per PSUM bank) -> strided ACT copy PSUM->SBUF restoring
  block order -> PE conv matmuls (2 accumulating fp32r matmuls per
  512-block group) -> DVE abs-sum reduce from PSUM -> acc[128, 24] -> DMA.

Measured on HW: ~35-40 us steady-state per core (HBM roofline ~35 us),
final-metric rel err ~2.3e-5 vs the f32 reference recurrence.
"""

import sys

if "/opt/trn_rl_repo" not in sys.path:
    sys.path.insert(0, "/opt/trn_rl_repo")

from contextlib import ExitStack

import numpy as np

import concourse.bacc as bacc
import concourse.tile as tile
from concourse import masks, mybir
from concourse import bass_utils

F32 = mybir.dt.float32
F32R = mybir.dt.float32r

B, T = 96, 131072
NCORES = 8
WPC = B // NCORES          # waveforms per core
P = 128                    # partitions / block size
U = T // (P * P)           # 8 sub-tiles per waveform
NBLK = T // P              # 1024 blocks per waveform
GRP = 512                  # output blocks per conv matmul group
NGRP = NBLK // GRP         # 2 groups per waveform
KT = 256                   # FIR taps

CUTOFF_FREQ = 4000.0
SAMPLE_RATE = 48000.0
FILTER_ORDER = 6

# Which dtype the conv matmuls run in: "f32r" (full speed) or "f32" (4x slower,
# bit-exact fp32).
CONV_DTYPE = "f32r"


def _butter_lowpass(order, wn):
    fs = 2.0
    warped = 2.0 * fs * np.tan(np.pi * wn / fs)
    m = np.arange(-order + 1, order, 2)
    p = -np.exp(1j * np.pi * m / (2 * order))
    p = warped * p
    k = warped ** order
    fs2 = 2.0 * fs
    pz = (fs2 + p) / (fs2 - p)
    kz = k * np.real(1.0 / np.prod(fs2 - p))
    b = (kz * np.poly(-np.ones(order))).real
    a = np.poly(pz).real
    return b.astype(np.float32), a.astype(np.float32)


def _impulse_response(n):
    """First n taps of the reference filter's impulse response, in float64
    (coefficients quantized to f32 exactly as the reference does)."""
    b32, a32 = _butter_lowpass(FILTER_ORDER, CUTOFF_FREQ / (0.5 * SAMPLE_RATE))
    b = b32.astype(np.float64)
    a = a32.astype(np.float64)
    order = len(b) - 1
    z = np.zeros(order)
    h = np.zeros(n)
    x = np.zeros(n)
    x[0] = 1.0
    b0, bi, ai = b[0], b[1:], a[1:]
    for t in range(n):
        y = b0 * x[t] + z[0]
        z = np.concatenate([z[1:], [0.0]]) + bi * x[t] - ai * y
        h[t] = y
    return h


def _h_matrices():
    """hm[c, s, r] = h[128 s + r - c] — the pre-transposed stationary operands
    (lhsT) for the two conv matmuls."""
    h = _impulse_response(KT)
    r = np.arange(P)[None, :]
    c = np.arange(P)[:, None]
    hm = np.zeros((P, 2, P), np.float32)
    for s in range(2):
        j = 128 * s + r - c
        valid = (j >= 0) & (j < KT)
        hm[:, s, :] = np.where(valid, h[np.clip(j, 0, KT - 1)], 0.0).astype(
            np.float32
        )
    return hm


def _build_program(unroll=1, stages="full"):
    """unroll > 1 replicates the whole body (same data) inside one NEFF —
    used only for steady-state throughput measurement in bench.py.
    stages: "full" | "dma" (DMA+subtract only) | "noconv" (through copies)."""
    conv_dt = F32R if CONV_DTYPE == "f32r" else F32
    nc = bacc.Bacc("TRN2", target_bir_lowering=False, debug=False)

    # output and target packed into one tensor so each waveform is ONE 1MB
    # DMA -> the subtract has a single DMA-semaphore wait (the TensorTensor
    # ISA struct only has room for one sync wait).
    x_both = nc.dram_tensor(
        "x_both", [WPC, 2, T], F32, kind="ExternalInput"
    ).ap()
    hmat = nc.dram_tensor("hmat", [P, 2, P], conv_dt, kind="ExternalInput").ap()
    zpad = nc.dram_tensor("zpad", [P, 1], conv_dt, kind="ExternalInput").ap()
    acc = nc.dram_tensor(
        "acc", [P, WPC * NGRP], F32, kind="ExternalOutput"
    ).ap()

    # Strip layout: partition p holds a contiguous 4KB run x[w, k, 1024p+f]
    # (one large DMA descriptor per partition per tensor). Block index of
    # 128-sample block u within partition p is B = 8p + u; the strided ACT
    # copy after the transpose restores natural block order in xt.
    FPW = T // P  # 1024 samples per partition per waveform
    xb_v = x_both.rearrange("w k (p f) -> w p k f", p=P, f=FPW)

    with ExitStack() as ctx:
        tc = ctx.enter_context(tile.TileContext(nc))
        singles = ctx.enter_context(tc.tile_pool(name="singles", bufs=1))
        stage = ctx.enter_context(tc.tile_pool(name="stage", bufs=2))
        dpool = ctx.enter_context(tc.tile_pool(name="dpool", bufs=2))
        xtp = ctx.enter_context(tc.tile_pool(name="xtp", bufs=2))
        tpsum = ctx.enter_context(
            tc.tile_pool(name="tpsum", bufs=4, space="PSUM")
        )
        ypsum = ctx.enter_context(
            tc.tile_pool(name="ypsum", bufs=2, space="PSUM")
        )

        # Constants are "pre-touched" by DVE copies so that consumers (PE)
        # only ever need the DVE semaphore they already wait on for data —
        # several ISA structs (TT, LW) have a single sync-wait slot.
        hmat_sb = singles.tile([P, 2, P], conv_dt)
        nc.sync.dma_start(hmat_sb, hmat)
        hmat_c = singles.tile([P, 2, P], conv_dt)
        nc.vector.tensor_copy(hmat_c, hmat_sb)
        ident_g = singles.tile([P, P], F32)
        masks.make_identity(nc, ident_g)
        ident = singles.tile([P, P], conv_dt)
        nc.vector.tensor_copy(ident, ident_g)
        acc_sb = singles.tile([P, WPC * NGRP], F32)

        h0 = hmat_c[:, 0, :]
        h1 = hmat_c[:, 1, :]

        for w0 in range(WPC * unroll):
            w = w0 % WPC
            big = stage.tile([P, 2, FPW], F32)
            nc.sync.dma_start(big, xb_v[w])

            d_t = dpool.tile([P, FPW], conv_dt)
            nc.vector.tensor_sub(d_t, big[:, 0], big[:, 1])

            if stages == "dma":
                for g in range(NGRP):
                    nc.vector.tensor_reduce(
                        acc_sb[:, w * NGRP + g : w * NGRP + g + 1],
                        d_t[:, GRP * g : GRP * (g + 1)],
                        axis=mybir.AxisListType.X,
                        op=mybir.AluOpType.add,
                        apply_absolute_value=True,
                    )
                continue

            # xt[:, 0] = zero pad; xt[:, 1 + B] = transposed block B
            xt = xtp.tile([P, 1 + NBLK], conv_dt)
            nc.sync.dma_start(xt[:, 0:1], zpad)
            # view of xt[:, 1:] with free dims [u (stride 1), p (stride 8)]
            xt_blk = xt[:, 1 : 1 + NBLK].rearrange(
                "c (p uu) -> c uu p", uu=U, p=P
            )
            for h in range(2):
                tp = tpsum.tile([P, 4, P], conv_dt)
                for j in range(4):
                    u = 4 * h + j
                    nc.tensor.transpose(
                        tp[:, j, :], d_t[:, P * u : P * (u + 1)], ident
                    )
                # psum [c, j, p] -> xt columns 1 + 8p + 4h + j
                nc.scalar.copy(xt_blk[:, 4 * h : 4 * h + 4, :], tp)

            if stages == "noconv":
                for g in range(NGRP):
                    nc.vector.tensor_reduce(
                        acc_sb[:, w * NGRP + g : w * NGRP + g + 1],
                        xt[:, 1 + GRP * g : 1 + GRP * (g + 1)],
                        axis=mybir.AxisListType.X,
                        op=mybir.AluOpType.add,
                        apply_absolute_value=True,
                    )
                continue

            for g in range(NGRP):
                yp = ypsum.tile([P, GRP], F32)
                rhs0 = xt[:, 1 + GRP * g : 1 + GRP * (g + 1)]
                rhs1 = xt[:, GRP * g : GRP * g + GRP]
                nc.tensor.matmul(yp, h0, rhs0, start=True, stop=False)
                nc.tensor.matmul(yp, h1, rhs1, start=False, stop=True)
                nc.vector.tensor_reduce(
                    acc_sb[:, w * NGRP + g : w * NGRP + g + 1],
                    yp,
                    axis=mybir.AxisListType.X,
                    op=mybir.AluOpType.add,
                    apply_absolute_value=True,
                )

        nc.sync.dma_start(acc, acc_sb)

    # Splits multi-sem waits into event semaphores (TRN2 allows only one
    # embedded wait per instruction), among other legalization passes.
    nc.compile()
    return nc


_CACHE = {}


def _get_program(unroll=1, stages="full"):
    key = ("nc", unroll, stages)
    if key not in _CACHE:
        _CACHE[key] = _build_program(unroll, stages)
    if "hmat" not in _CACHE:
        _CACHE["hmat"] = _h_matrices()
    return _CACHE[key], _CACHE["hmat"]


def _run(output, target, trace=False):
    """Run on the 8 NeuronCores; returns (result_scalar, BassKernelResults)."""
    nc, hm = _get_program()
    output = np.asarray(output, dtype=np.float32)
    target = np.asarray(target, dtype=np.float32)
    both = np.stack([output, target], axis=1)  # [B, 2, T] contiguous
    in_maps = []
    for i in range(NCORES):
        sl = slice(i * WPC, (i + 1) * WPC)
        in_maps.append(
            {
                "x_both": np.ascontiguousarray(both[sl]),
                "hmat": hm,
                "zpad": np.zeros((P, 1), np.float32),
            }
        )
    res = bass_utils.run_bass_kernel_spmd(
        nc, in_maps, core_ids=list(range(NCORES)), trace=trace
    )
    total = 0.0
    for r in res.results:
        total += r["acc"].astype(np.float64).sum()
    value = np.float32(total / (B * T))
    return value, res


def kernel(output, target):
    value, _ = _run(output, target)
    return np.asarray(value, dtype=np.float32)
